# revision 25
# baseline (speedup 1.0000x reference)
"""Self-contained Trainium2 kernel for nn_DynamicCrossAttention_40286793236903.

kernel(**inputs) takes the FULL inputs (as produced by setup_inputs) and
returns the FULL [4, 256, 64, 64] float32 output.

Sharding: pure data parallel over (batch, image-half): core ci handles
sample b=ci//2, output rows 32*(ci%2)..32*(ci%2)+31. One SPMD Bass program
runs on all 8 cores; all per-core variation is carried in the input data.

Pipeline per core (2048 output pixels = 4 quarters of 512):
  1. bilinear-upsample template half (+halo) -> combined [512ch, 34, 66]
  2. offsets+mask 3x3 conv, per quarter q -> om[32q:32q+32, 512]
     (rows 32q+0..8 = dy taps, +9..17 = dx, +18..26 = mask)
  3. per half: index math (clamp to [-1, 64.996], floor/frac), quad-table
     idx = 66*(y0+1)+(x0+1), idx16 wrap for dma_gather, corner weights
     wq = bilinear products * sigmoid(mask)
  4. main loop per (quarter, tap): dma_gather 512 quad rows (2KB each: the
     4 bilinear corners, zero-padded at borders -> no validity masks);
     blend per 128-px block: chains of per-partition-scalar mult-adds
     spread over ACT (starts), DVE (fused mult-adds), GpSimd (pair sums,
     delayed one tap so they never stall the next gather's emission);
     PE-transpose to channel-major ST; einsum og=0 accumulated inline,
     og=1 as a per-quarter tail.
"""
import numpy as np
from contextlib import ExitStack

import concourse.bass as bass
import concourse.mybir as mybir
import concourse.tile as tile
from concourse import bacc
from concourse.bass import AP
from concourse.bass_utils import run_bass_kernel_spmd
from concourse.masks import make_identity

F32 = mybir.dt.float32
BF16 = mybir.dt.bfloat16
I32 = mybir.dt.int32
I16 = mybir.dt.int16
ALU = mybir.AluOpType
ACTF = mybir.ActivationFunctionType

TAPS = [(ky, kx) for ky in (-1, 0, 1) for kx in (-1, 0, 1)]
CLAMP_LO = -1.0
CLAMP_HI = 64.99609375

_NC_CACHE = {}
LAST_RESULT = None


def build_nc():
    nc = bacc.Bacc(None, target_bir_lowering=False, num_swdge_queues=4)

    tplp = nc.dram_tensor('tplp', [256, 23 * 32], BF16, kind='ExternalInput')
    srch66 = nc.dram_tensor('srch66', [256, 34 * 66], BF16, kind='ExternalInput')
    qtab = nc.dram_tensor('qtab', [4356, 1024], BF16, kind='ExternalInput')
    wpack = nc.dram_tensor('wpack', [128, 4 * 9 * 32], BF16, kind='ExternalInput')
    dwpack = nc.dram_tensor('dwpack', [128, 9 * 2 * 2 * 128], BF16, kind='ExternalInput')
    basei = nc.dram_tensor('basei', [128, 512], F32, kind='ExternalInput')
    mcomb = nc.dram_tensor('mcomb', [128, 4 * 9], F32, kind='ExternalInput')
    bias_om = nc.dram_tensor('bias_om', [128, 1], F32, kind='ExternalInput')
    bias_out = nc.dram_tensor('bias_out', [256, 1], F32, kind='ExternalInput')
    out = nc.dram_tensor('out', [256, 2048], F32, kind='ExternalOutput')

    with tile.TileContext(nc) as tc, ExitStack() as ctx:
        sb = ctx.enter_context(tc.tile_pool(name='sb', bufs=1))
        sbm = ctx.enter_context(tc.tile_pool(name='sbm', bufs=1))
        sbt = ctx.enter_context(tc.tile_pool(name='sbt', bufs=2))
        sba = ctx.enter_context(tc.tile_pool(name='sba', bufs=4))
        sbg = ctx.enter_context(tc.tile_pool(name='sbg', bufs=6))
        gpool = ctx.enter_context(tc.tile_pool(name='gpool', bufs=8))
        spool = ctx.enter_context(tc.tile_pool(name='spool', bufs=3))
        stpool = ctx.enter_context(tc.tile_pool(name='stpool', bufs=2))

        ident = sb.tile([128, 128], BF16, tag='ident', name='ident')
        make_identity(nc, ident[:])
        identf = sb.tile([128, 128], F32, tag='identf', name='identf')
        make_identity(nc, identf[:])

        wp = sb.tile([128, 4 * 9 * 32], BF16, tag='wp', name='wp')
        nc.sync.dma_start(wp[:], wpack[:])
        dw = sb.tile([128, 9 * 2 * 2 * 128], BF16, tag='dw', name='dw')
        nc.sync.dma_start(dw[:], dwpack[:])
        base_sb = sb.tile([128, 512], F32, tag='base', name='base')
        nc.sync.dma_start(base_sb[:], basei[:])
        mc_sb = sb.tile([128, 4 * 9], F32, tag='mcomb', name='mcomb')
        nc.sync.dma_start(mc_sb[:], mcomb[:])
        bom_sb = sb.tile([128, 1], F32, tag='bom', name='bom')
        nc.sync.dma_start(bom_sb[:], bias_om[:])
        bout_sb = sb.tile([128, 2], F32, tag='bout', name='bout')
        nc.sync.dma_start(bout_sb[:], bias_out[:].rearrange('(g p) o -> p (g o)', g=2))

        # persistent tiles for the idx/weight pipeline
        om = sb.tile([128, 512], F32, tag='om', name='om')
        sg = sb.tile([128, 512], F32, tag='sg', name='sg')
        idxf = sb.tile([9, 2048], F32, tag='idxf', name='idxf')
        idx16 = sb.tile([128, 9 * 128], I16, tag='idx16', name='idx16')
        tsb = sb.tile([128, 16 * 9], F32, tag='tsb', name='tsb')
        tsb3 = tsb[:].rearrange('p (b t) -> p b t', b=16)
        wq = sb.tile([128, 16 * 4 * 9], F32, tag='wq', name='wq')
        wq4 = wq[:].rearrange('p (b j t) -> p b j t', b=16, j=4)
        taW = sb.tile([128, 16 * 45], F32, tag='taW', name='taW')
        ta4 = taW[:].rearrange('p (b c) -> p b c', b=16)
        iw = sb.tile([128, 16 * 2 * 9], F32, tag='iw', name='iw')
        iw4 = iw[:].rearrange('p (b j t) -> p b j t', b=16, j=2)
        q1t = sb.tile([128, 16 * 9], F32, tag='q1t', name='q1t')
        q13 = q1t[:].rearrange('p (b t) -> p b t', b=16)
        q2t = sb.tile([128, 16 * 9], F32, tag='q2t', name='q2t')
        q23 = q2t[:].rearrange('p (b t) -> p b t', b=16)

        def mtile(tag, dt=F32):
            return sbm.tile([128, 512], dt, tag=tag, name=tag)
        P = mtile('P')
        T32 = mtile('T32', I32)
        Tf = mtile('Tf')
        Gg = mtile('Gg')

        with tc.tile_pool(name='convsb', bufs=1) as convsb, \
             tc.tile_pool(name='psA', bufs=1, space='PSUM') as psA, \
             tc.tile_pool(name='psI', bufs=2, space='PSUM') as psI, \
             tc.tile_pool(name='psT', bufs=2, space='PSUM') as psT, \
             tc.tile_pool(name='psW', bufs=2, space='PSUM') as psW:
            # ---- stage 1: upsample template + build combined [512ch, 34, 66]
            # (the fixed-scale mults run on GpSimd, idle at this point)
            comb = []
            with tc.tile_pool(name='upsb', bufs=1) as upsb:
                for cg in range(2):
                    tp = upsb.tile([128, 23 * 32], BF16, tag=f'tp{cg}', name=f'tp{cg}')
                    nc.sync.dma_start(tp[:], tplp[128 * cg:128 * (cg + 1), :])
                    tp3 = tp[:].rearrange('p (r w) -> p r w', r=23)
                    V = upsb.tile([128, 34 * 32], BF16, tag=f'vt{cg}', name=f'vt{cg}')
                    V3 = V[:].rearrange('p (r w) -> p r w', r=34)
                    tmp = upsb.tile([128, 34 * 32], BF16, tag=f'ut{cg}', name=f'ut{cg}')
                    tmp3 = tmp[:].rearrange('p (r w) -> p r w', r=34)
                    nc.vector.tensor_scalar_mul(tmp3[:, 0:16, :], tp3[:, 2:18, :], 0.25)
                    nc.gpsimd.tensor_scalar(V3[:, 2:34:2, :], tp3[:, 1:17, :], 0.75, None, ALU.mult)
                    nc.vector.tensor_tensor(V3[:, 2:34:2, :], V3[:, 2:34:2, :], tmp3[:, 0:16, :], ALU.add)
                    nc.vector.tensor_scalar_mul(tmp3[:, 0:16, :], tp3[:, 1:17, :], 0.75)
                    nc.gpsimd.tensor_scalar(V3[:, 1:33:2, :], tp3[:, 0:16, :], 0.25, None, ALU.mult)
                    nc.vector.tensor_tensor(V3[:, 1:33:2, :], V3[:, 1:33:2, :], tmp3[:, 0:16, :], ALU.add)
                    nc.vector.tensor_scalar_mul(tmp3[:, 0:1, :], tp3[:, 20:21, :], 0.25)
                    nc.gpsimd.tensor_scalar(V3[:, 0:1, :], tp3[:, 19:20, :], 0.75, None, ALU.mult)
                    nc.vector.tensor_tensor(V3[:, 0:1, :], V3[:, 0:1, :], tmp3[:, 0:1, :], ALU.add)
                    nc.vector.tensor_scalar_mul(tmp3[:, 0:1, :], tp3[:, 22:23, :], 0.75)
                    nc.gpsimd.tensor_scalar(V3[:, 33:34, :], tp3[:, 21:22, :], 0.25, None, ALU.mult)
                    nc.vector.tensor_tensor(V3[:, 33:34, :], V3[:, 33:34, :], tmp3[:, 0:1, :], ALU.add)
                    cb = convsb.tile([128, 34 * 66], BF16, tag=f'comb{cg}', name=f'comb{cg}')
                    cb3 = cb[:].rearrange('p (r w) -> p r w', r=34)
                    nc.gpsimd.memset(cb[:], 0.0)
                    h3 = tmp3
                    nc.vector.tensor_scalar_mul(h3[:, :, 0:31], V3[:, :, 1:32], 0.75)
                    nc.gpsimd.tensor_scalar(cb3[:, :, 3:65:2], V3[:, :, 0:31], 0.25, None, ALU.mult)
                    nc.vector.tensor_tensor(cb3[:, :, 3:65:2], cb3[:, :, 3:65:2], h3[:, :, 0:31], ALU.add)
                    nc.vector.tensor_scalar_mul(h3[:, :, 0:31], V3[:, :, 1:32], 0.25)
                    nc.gpsimd.tensor_scalar(cb3[:, :, 2:64:2], V3[:, :, 0:31], 0.75, None, ALU.mult)
                    nc.vector.tensor_tensor(cb3[:, :, 2:64:2], cb3[:, :, 2:64:2], h3[:, :, 0:31], ALU.add)
                    nc.vector.tensor_copy(cb3[:, :, 1:2], V3[:, :, 0:1])
                    nc.vector.tensor_copy(cb3[:, :, 64:65], V3[:, :, 31:32])
                    comb.append(cb)
                for cg in range(2):
                    cb = convsb.tile([128, 34 * 66], BF16, tag=f'comb{cg+2}', name=f'comb{cg+2}')
                    nc.sync.dma_start(cb[:], srch66[128 * cg:128 * (cg + 1), :])
                    comb.append(cb)

            # ---- stage 2+3: per-quarter conv; per-half idx/wq chain ----
            wp4 = wp[:].rearrange('p (g t m) -> p g t m', g=4, t=9)
            for q in range(4):
                pomq = psA.tile([32, 512], F32, tag=f'pom{q % 2}', name=f'pom{q % 2}')
                first = True
                for gi, g in enumerate((2, 3, 0, 1)):
                    cb3 = comb[g][:].rearrange('p (r w) -> p r w', r=34)
                    for t, (ky, kx) in enumerate(TAPS):
                        rhs = cb3[:, 8 * q + 1 + ky: 8 * q + 9 + ky, 1 + kx: 65 + kx]
                        nc.tensor.matmul(
                            pomq[:], wp4[:, g, t, :], rhs,
                            start=first, stop=(gi == 3 and t == 8),
                            tile_position=(0, 0))
                        first = False
                ps = slice(32 * q, 32 * q + 32)
                nc.scalar.activation(om[ps, :], pomq[:], ACTF.Identity,
                                     bias=bom_sb[ps, :], scale=1.0)
                nc.scalar.activation(sg[ps, :], om[ps, :], ACTF.Sigmoid)

                if q % 2 == 0:
                    continue
                hq = q // 2            # half index: quarters 2hq, 2hq+1
                hs = slice(64 * hq, 64 * hq + 64)
                # index math for this half
                nc.vector.tensor_tensor(P[hs, :], om[hs, :], base_sb[hs, :], ALU.add)
                nc.vector.tensor_scalar(P[hs, :], P[hs, :], CLAMP_HI, CLAMP_LO,
                                        ALU.min, ALU.max)
                nc.vector.tensor_copy(T32[hs, :], P[hs, :])
                nc.vector.tensor_copy(Tf[hs, :], T32[hs, :])
                nc.vector.tensor_tensor(Gg[hs, :], Tf[hs, :], P[hs, :], ALU.is_gt)
                nc.vector.tensor_tensor(Tf[hs, :], Tf[hs, :], Gg[hs, :], ALU.subtract)
                nc.vector.tensor_tensor(P[hs, :], P[hs, :], Tf[hs, :], ALU.subtract)
                Wf = P
                for qq in (2 * hq, 2 * hq + 1):
                    qs = slice(32 * qq, 32 * qq + 32)
                    # idxf: 66*(y0+1) + (x0+1)
                    pidx = psI.tile([9, 512], F32, name='pidx')
                    nc.tensor.matmul(pidx[:], mc_sb[qs, 9 * qq:9 * (qq + 1)],
                                     Tf[qs, :], start=True, stop=True,
                                     tile_position=(32 * qq, 0))
                    nc.vector.tensor_scalar(idxf[:, 512 * qq:512 * (qq + 1)], pidx[:],
                                            67.0, None, ALU.add)
                    # wq transposes for this quarter's four 128-px blocks
                    for cc in range(4):
                        bb = 4 * qq + cc
                        pt = psW.tile([128, 45], F32, name='ptw')
                        idq18 = identf[32 * qq:32 * qq + 18, 32 * qq:32 * qq + 18]
                        idq27 = identf[32 * qq:32 * qq + 27, 32 * qq:32 * qq + 27]
                        nc.tensor.transpose(
                            pt[:, 0:18],
                            Wf[32 * qq:32 * qq + 18, 128 * cc:128 * (cc + 1)],
                            idq18, tile_position=(32 * qq, 0))
                        nc.tensor.transpose(
                            pt[:, 18:45],
                            sg[32 * qq:32 * qq + 27, 128 * cc:128 * (cc + 1)],
                            idq27, tile_position=(32 * qq, 0))
                        nc.vector.tensor_copy(ta4[:, bb, :], pt[:])
                # idx16 wrap for this half
                for bb in range(8 * hq, 8 * hq + 8):
                    pT = psT.tile([128, 9], F32, tag='pTm', name='pT')
                    nc.tensor.transpose(pT[:], idxf[:, 128 * bb:128 * (bb + 1)],
                                        identf[0:9, 0:9], tile_position=(0, 0))
                    nc.vector.tensor_copy(tsb3[:, bb, :], pT[:])
                for a in range(8):
                    pWm = psT.tile([16, 72], F32, tag='pTm', name='pWm')
                    nc.tensor.matmul(pWm[:], identf[:, 16 * a:16 * (a + 1)],
                                     tsb[:, 72 * hq:72 * (hq + 1)],
                                     start=True, stop=True)
                    dsta = AP(idx16[:].tensor, idx16[:].offset + 64 * hq + a,
                              [[9 * 128, 16], [8, 8], [128, 9]])
                    nc.vector.tensor_copy(
                        dsta, pWm[:].rearrange('p (b t) -> p b t', b=8))
                # replicate idx rows 0:16 across all 128 partitions
                for (dst, src, npart) in ((16, 0, 16), (32, 0, 32), (64, 0, 64)):
                    d = AP(idx16[:].tensor, idx16[:].offset + dst * 9 * 128 + 64 * hq,
                           [[9 * 128, npart], [128, 9], [1, 64]])
                    s = AP(idx16[:].tensor, idx16[:].offset + src * 9 * 128 + 64 * hq,
                           [[9 * 128, npart], [128, 9], [1, 64]])
                    nc.sync.dma_start(d, s)
                # corner-weight products for this half's 8 blocks
                bs = slice(8 * hq, 8 * hq + 8)
                wy_ap = ta4[:, bs, 0:9]
                wx_ap = ta4[:, bs, 9:18]
                mk_ap = ta4[:, bs, 36:45]
                nc.vector.tensor_scalar(iw4[:, bs, 0, :], wy_ap, -1.0, 1.0,
                                        ALU.mult, ALU.add)
                nc.vector.tensor_scalar(iw4[:, bs, 1, :], wx_ap, -1.0, 1.0,
                                        ALU.mult, ALU.add)
                nc.vector.tensor_tensor(q13[:, bs, :], iw4[:, bs, 0, :], mk_ap, ALU.mult)
                nc.vector.tensor_tensor(q23[:, bs, :], wy_ap, mk_ap, ALU.mult)
                nc.vector.tensor_tensor(wq4[:, bs, 0, :], q13[:, bs, :],
                                        iw4[:, bs, 1, :], ALU.mult)
                nc.vector.tensor_tensor(wq4[:, bs, 1, :], q13[:, bs, :], wx_ap, ALU.mult)
                nc.vector.tensor_tensor(wq4[:, bs, 2, :], q23[:, bs, :],
                                        iw4[:, bs, 1, :], ALU.mult)
                nc.vector.tensor_tensor(wq4[:, bs, 3, :], q23[:, bs, :], wx_ap, ALU.mult)

        # ---- stages 4-7 ----
        inap = AP(qtab[:].tensor, 0, [[1024, 4356], [1, 1024]])
        dw4 = dw[:].rearrange('p (k g o c) -> p k g o c', k=9, g=2, o=2)

        def chain_A(g3, sblk, S3, bb, t):
            # 1 ACT start + 3 DVE fused mult-adds
            m = sba.tile([128, 256], BF16, tag='m', name='m')
            nc.scalar.activation(m[:], g3[:, sblk, 0:256], ACTF.Identity,
                                 scale=wq4[:, bb, 0, t:t + 1])
            nc.vector.scalar_tensor_tensor(
                m[:], g3[:, sblk, 256:512], wq4[:, bb, 1, t:t + 1], m[:],
                ALU.mult, ALU.add)
            nc.vector.scalar_tensor_tensor(
                m[:], g3[:, sblk, 512:768], wq4[:, bb, 2, t:t + 1], m[:],
                ALU.mult, ALU.add)
            nc.vector.scalar_tensor_tensor(
                S3[:, sblk, :], g3[:, sblk, 768:1024], wq4[:, bb, 3, t:t + 1], m[:],
                ALU.mult, ALU.add)

        def chain_B_front(g3, sblk, bb, t):
            # 2 ACT starts + 2 DVE fused mult-adds; the pair-sum is deferred
            # to GpSimd (emitted after the next gather's descriptors)
            m1 = sbg.tile([128, 256], BF16, tag='mb1', name='mb1')
            m2 = sbg.tile([128, 256], BF16, tag='mb2', name='mb2')
            nc.scalar.activation(m1[:], g3[:, sblk, 0:256], ACTF.Identity,
                                 scale=wq4[:, bb, 0, t:t + 1])
            nc.vector.scalar_tensor_tensor(
                m1[:], g3[:, sblk, 256:512], wq4[:, bb, 1, t:t + 1], m1[:],
                ALU.mult, ALU.add)
            nc.scalar.activation(m2[:], g3[:, sblk, 512:768], ACTF.Identity,
                                 scale=wq4[:, bb, 2, t:t + 1])
            nc.vector.scalar_tensor_tensor(
                m2[:], g3[:, sblk, 768:1024], wq4[:, bb, 3, t:t + 1], m2[:],
                ALU.mult, ALU.add)
            return m1, m2

        with tc.tile_pool(name='psQ', bufs=2, space='PSUM') as psQ, \
             tc.tile_pool(name='psO', bufs=2, space='PSUM') as psO, \
             tc.tile_pool(name='psO1', bufs=2, space='PSUM') as psO1:
            for qo in range(4):
                ST = stpool.tile([128, 18 * 512], BF16, tag='ST', name='ST')
                ST3 = ST[:].rearrange('p (k n) -> p k n', k=18)
                po0 = psO.tile([128, 512], F32, tag='po0', name='po0')
                gps_pend = []   # deferred GpSimd pair-sums
                Ss = {}

                def do_transposes(ptap):
                    pS3 = Ss[ptap]
                    pq = psQ.tile([128, 1024], BF16, name='pq')
                    for cg in range(2):
                        for sblk in range(4):
                            nc.tensor.transpose(
                                pq[:, 512 * cg + 128 * sblk:512 * cg + 128 * (sblk + 1)],
                                pS3[:, sblk, 128 * cg:128 * (cg + 1)], ident[:])
                    nc.scalar.activation(
                        ST3[:, 2 * ptap:2 * ptap + 2, :],
                        pq[:].rearrange('p (c n) -> p c n', c=2), ACTF.Identity)
                    for cg in range(2):
                        nc.tensor.matmul(
                            po0[:], dw4[:, ptap, cg, 0, :], ST3[:, 2 * ptap + cg, :],
                            start=(ptap == 0 and cg == 0), stop=(ptap == 8 and cg == 1))

                for t in range(9):
                    g = gpool.tile([128, 4, 1024], BF16, tag='gt', name='gt')
                    nc.gpsimd.dma_gather(
                        out_ap=g[:], in_ap=inap,
                        idxs_ap=idx16[:, 128 * t + 32 * qo:128 * t + 32 * (qo + 1)],
                        num_idxs=512, num_idxs_reg=512,
                        elem_size=1024, elem_step=1024,
                        single_packet=False,
                        queue_num=(9 * qo + t) % 4)
                    g3 = g[:]
                    for (dst, m1, m2) in gps_pend:
                        nc.gpsimd.tensor_tensor(dst, m1[:], m2[:], ALU.add)
                    gps_pend = []
                    S = spool.tile([128, 4, 256], BF16, tag='S', name='S')
                    S3 = S[:]
                    Ss[t] = S3
                    for sblk in range(4):
                        bb = 4 * qo + sblk
                        if sblk >= 2 or (sblk == 1 and t % 2 == 0):
                            m1, m2 = chain_B_front(g3, sblk, bb, t)
                            gps_pend.append((S3[:, sblk, :], m1, m2))
                        else:
                            chain_A(g3, sblk, S3, bb, t)
                    if t > 0:
                        do_transposes(t - 1)
                for (dst, m1, m2) in gps_pend:
                    nc.gpsimd.tensor_tensor(dst, m1[:], m2[:], ALU.add)
                gps_pend = []
                do_transposes(8)
                osb = sbt.tile([128, 512], F32, tag='osb', name='osb')
                nc.scalar.activation(osb[:], po0[:], ACTF.Identity,
                                     bias=bout_sb[:, 0:1], scale=1.0)
                nc.sync.dma_start(out[0:128, 512 * qo:512 * (qo + 1)], osb[:])
                po = psO1.tile([128, 512], F32, name='po')
                for t in range(9):
                    for cg in range(2):
                        nc.tensor.matmul(
                            po[:], dw4[:, t, cg, 1, :], ST3[:, 2 * t + cg, :],
                            start=(t == 0 and cg == 0), stop=(t == 8 and cg == 1))
                osb1 = sbt.tile([128, 512], F32, tag='osb1', name='osb1')
                nc.scalar.activation(osb1[:], po[:], ACTF.Identity,
                                     bias=bout_sb[:, 1:2], scale=1.0)
                nc.sync.dma_start(out[128:256, 512 * qo:512 * (qo + 1)], osb1[:])

    nc.compile()
    return nc


def _bf16(x):
    import ml_dtypes
    return np.asarray(x, dtype=np.float32).astype(ml_dtypes.bfloat16)


def prep_sample(inputs, b):
    """Per-sample (shared by both h-halves) heavy prep: the quad table."""
    sf = np.ascontiguousarray(np.asarray(inputs['search_feat'][b], dtype=np.float32))
    P = np.zeros((67, 67, 256), np.float32)
    P[1:65, 1:65] = sf.transpose(1, 2, 0)
    Q = np.concatenate([P[:66, :66], P[:66, 1:67], P[1:67, :66], P[1:67, 1:67]],
                       axis=-1)
    return _bf16(Q.reshape(4356, 1024))


def prep_core_inputs(inputs, b, h, qtab):
    tf = np.ascontiguousarray(np.asarray(inputs['template_feat'][b], dtype=np.float32))
    sf = np.ascontiguousarray(np.asarray(inputs['search_feat'][b], dtype=np.float32))
    offset_w = np.asarray(inputs['offset_w'], dtype=np.float32)
    offset_b = np.asarray(inputs['offset_b'], dtype=np.float32)
    mask_w = np.asarray(inputs['mask_w'], dtype=np.float32)
    mask_b = np.asarray(inputs['mask_b'], dtype=np.float32)
    deform_w = np.asarray(inputs['deform_w'], dtype=np.float32)
    deform_b = np.asarray(inputs['deform_b'], dtype=np.float32)

    tplp = np.zeros((256, 23, 32), np.float32)
    for j in range(19):
        tplp[:, j] = tf[:, min(max(16 * h - 1 + j, 0), 31)]
    if h == 0:
        tplp[:, 21] = tf[:, 15]
        tplp[:, 22] = tf[:, 16]
    else:
        tplp[:, 19] = tf[:, 15]
        tplp[:, 20] = tf[:, 16]

    srch66 = np.zeros((256, 34, 66), np.float32)
    for i in range(34):
        r = 32 * h - 1 + i
        if 0 <= r <= 63:
            srch66[:, i, 1:65] = sf[:, r]

    wpack = np.zeros((128, 4, 9, 32), np.float32)
    for g in range(4):
        for t, (ky, kx) in enumerate(TAPS):
            cs = slice(128 * g, 128 * (g + 1))
            wpack[:, g, t, 0:9] = offset_w[0::2, cs, ky + 1, kx + 1].T
            wpack[:, g, t, 9:18] = offset_w[1::2, cs, ky + 1, kx + 1].T
            if ky == 0 and kx == 0:
                wpack[:, g, t, 18:27] = mask_w[:, cs, 0, 0].T
    wk = deform_w.reshape(256, 256, 3, 3)
    dwp = np.zeros((128, 9, 2, 2, 128), np.float32)
    for t in range(9):
        ky, kx = TAPS[t]
        for cg in range(2):
            for og in range(2):
                dwp[:, t, cg, og, :] = wk[128 * og:128 * (og + 1),
                                          128 * cg:128 * (cg + 1), ky + 1, kx + 1].T

    basei = np.zeros((128, 512), np.float32)
    col = np.arange(512)
    for q in range(4):
        for m in range(9):
            basei[32 * q + m] = 32 * h + 8 * q + col // 64 + TAPS[m][0]
            basei[32 * q + 9 + m] = col % 64 + TAPS[m][1]

    mcomb = np.zeros((128, 4, 9), np.float32)
    for q in range(4):
        for t in range(9):
            mcomb[32 * q + t, q, t] = 66.0
            mcomb[32 * q + 9 + t, q, t] = 1.0

    bias_om = np.zeros((128, 1), np.float32)
    for q in range(4):
        bias_om[32 * q + 0:32 * q + 9, 0] = offset_b[0::2]
        bias_om[32 * q + 9:32 * q + 18, 0] = offset_b[1::2]
        bias_om[32 * q + 18:32 * q + 27, 0] = mask_b

    return {
        'tplp': _bf16(tplp.reshape(256, 23 * 32)),
        'srch66': _bf16(srch66.reshape(256, 34 * 66)),
        'qtab': qtab,
        'wpack': _bf16(wpack.reshape(128, 4 * 9 * 32)),
        'dwpack': _bf16(dwp.reshape(128, 9 * 2 * 2 * 128)),
        'basei': basei,
        'mcomb': mcomb.reshape(128, 4 * 9),
        'bias_om': bias_om,
        'bias_out': deform_b.reshape(256, 1).astype(np.float32),
    }


def kernel(**inputs):
    key = 'v41'
    if key not in _NC_CACHE:
        _NC_CACHE[key] = build_nc()
    nc = _NC_CACHE[key]
    qtabs = [prep_sample(inputs, b) for b in range(4)]
    in_maps = [prep_core_inputs(inputs, ci // 2, ci % 2, qtabs[ci // 2])
               for ci in range(8)]
    res = run_bass_kernel_spmd(nc, in_maps, core_ids=list(range(8)))
    global LAST_RESULT
    LAST_RESULT = res
    out = np.zeros((4, 256, 64, 64), np.float32)
    for ci in range(8):
        b, h = ci // 2, ci % 2
        out[b][:, 32 * h:32 * h + 32, :] = res.results[ci]['out'].reshape(256, 32, 64)
    return out


# revision 30
# speedup vs baseline: 3.1586x; 3.1586x over previous
"""Self-contained Trainium2 kernel for nn_DynamicCrossAttention_40286793236903.

kernel(**inputs) takes the FULL inputs (as produced by setup_inputs) and
returns the FULL [4, 256, 64, 64] float32 output.

Sharding: pure data parallel over (batch, image-half): core ci handles
sample b=ci//2, output rows 32*(ci%2)..32*(ci%2)+31. One SPMD Bass program
runs on all 8 cores; all per-core variation is carried in the input data.

Pipeline per core (2048 output pixels = 4 quarters of 512):
  1. bilinear-upsample template half (+halo) -> combined [512ch, 34, 66]
  2. offsets+mask 3x3 conv, per quarter q -> om[32q:32q+32, 512]
     (rows 32q+0..8 = dy taps, +9..17 = dx, +18..26 = mask)
  3. per half: index math (clamp to [-1, 64.996], floor/frac), quad-table
     idx = 66*(y0+1)+(x0+1), idx16 wrap for dma_gather, corner weights
     wq = bilinear products * sigmoid(mask)
  4. main loop per (quarter, tap): dma_gather 512 quad rows (2KB each: the
     4 bilinear corners, zero-padded at borders -> no validity masks);
     blend per 128-px block: chains of per-partition-scalar mult-adds
     spread over ACT (starts), DVE (fused mult-adds), GpSimd (pair sums,
     delayed one tap so they never stall the next gather's emission);
     PE-transpose to channel-major ST; einsum og=0 accumulated inline,
     og=1 as a per-quarter tail.
"""
import numpy as np
from contextlib import ExitStack

import concourse.bass as bass
import concourse.mybir as mybir
import concourse.tile as tile
from concourse import bacc
from concourse.bass import AP
from concourse.bass_utils import run_bass_kernel_spmd
from concourse.masks import make_identity

F32 = mybir.dt.float32
BF16 = mybir.dt.bfloat16
I32 = mybir.dt.int32
I16 = mybir.dt.int16
ALU = mybir.AluOpType
ACTF = mybir.ActivationFunctionType

TAPS = [(ky, kx) for ky in (-1, 0, 1) for kx in (-1, 0, 1)]
CLAMP_LO = -1.0
CLAMP_HI = 64.99609375

_NC_CACHE = {}
LAST_RESULT = None


def build_nc():
    nc = bacc.Bacc(None, target_bir_lowering=False, num_swdge_queues=4)

    tplp = nc.dram_tensor('tplp', [256, 23 * 32], BF16, kind='ExternalInput')
    srch66 = nc.dram_tensor('srch66', [256, 34 * 66], BF16, kind='ExternalInput')
    qtab = nc.dram_tensor('qtab', [4356, 1024], BF16, kind='ExternalInput')
    wpack = nc.dram_tensor('wpack', [128, 4 * 9 * 32], BF16, kind='ExternalInput')
    dwpack = nc.dram_tensor('dwpack', [128, 9 * 2 * 2 * 128], BF16, kind='ExternalInput')
    basei = nc.dram_tensor('basei', [128, 512], F32, kind='ExternalInput')
    mcomb = nc.dram_tensor('mcomb', [128, 4 * 9], F32, kind='ExternalInput')
    bias_om = nc.dram_tensor('bias_om', [128, 1], F32, kind='ExternalInput')
    bias_out = nc.dram_tensor('bias_out', [256, 1], F32, kind='ExternalInput')
    out = nc.dram_tensor('out', [256, 2048], F32, kind='ExternalOutput')

    with tile.TileContext(nc) as tc, ExitStack() as ctx:
        sb = ctx.enter_context(tc.tile_pool(name='sb', bufs=1))
        sbm = ctx.enter_context(tc.tile_pool(name='sbm', bufs=1))
        sbt = ctx.enter_context(tc.tile_pool(name='sbt', bufs=2))
        sba = ctx.enter_context(tc.tile_pool(name='sba', bufs=4))
        sbg = ctx.enter_context(tc.tile_pool(name='sbg', bufs=6))
        gpool = ctx.enter_context(tc.tile_pool(name='gpool', bufs=8))
        spool = ctx.enter_context(tc.tile_pool(name='spool', bufs=3))
        stpool = ctx.enter_context(tc.tile_pool(name='stpool', bufs=2))

        ident = sb.tile([128, 128], BF16, tag='ident', name='ident')
        make_identity(nc, ident[:])
        identf = sb.tile([128, 128], F32, tag='identf', name='identf')
        make_identity(nc, identf[:])

        wp = sb.tile([128, 4 * 9 * 32], BF16, tag='wp', name='wp')
        nc.sync.dma_start(wp[:], wpack[:])
        dw = sb.tile([128, 9 * 2 * 2 * 128], BF16, tag='dw', name='dw')
        nc.sync.dma_start(dw[:], dwpack[:])
        base_sb = sb.tile([128, 512], F32, tag='base', name='base')
        nc.sync.dma_start(base_sb[:], basei[:])
        mc_sb = sb.tile([128, 4 * 9], F32, tag='mcomb', name='mcomb')
        nc.sync.dma_start(mc_sb[:], mcomb[:])
        bom_sb = sb.tile([128, 1], F32, tag='bom', name='bom')
        nc.sync.dma_start(bom_sb[:], bias_om[:])
        bout_sb = sb.tile([128, 2], F32, tag='bout', name='bout')
        nc.sync.dma_start(bout_sb[:], bias_out[:].rearrange('(g p) o -> p (g o)', g=2))

        # persistent tiles for the idx/weight pipeline
        om = sb.tile([128, 512], F32, tag='om', name='om')
        sg = sb.tile([128, 512], F32, tag='sg', name='sg')
        idxf = sb.tile([9, 2048], F32, tag='idxf', name='idxf')
        idx16 = sb.tile([128, 9 * 128], I16, tag='idx16', name='idx16')
        tsb = sb.tile([128, 16 * 9], F32, tag='tsb', name='tsb')
        tsb3 = tsb[:].rearrange('p (b t) -> p b t', b=16)
        wq = sb.tile([128, 16 * 4 * 9], F32, tag='wq', name='wq')
        wq4 = wq[:].rearrange('p (b j t) -> p b j t', b=16, j=4)
        taW = sb.tile([128, 16 * 45], F32, tag='taW', name='taW')
        ta4 = taW[:].rearrange('p (b c) -> p b c', b=16)
        iw = sb.tile([128, 16 * 2 * 9], F32, tag='iw', name='iw')
        iw4 = iw[:].rearrange('p (b j t) -> p b j t', b=16, j=2)
        q1t = sb.tile([128, 16 * 9], F32, tag='q1t', name='q1t')
        q13 = q1t[:].rearrange('p (b t) -> p b t', b=16)
        q2t = sb.tile([128, 16 * 9], F32, tag='q2t', name='q2t')
        q23 = q2t[:].rearrange('p (b t) -> p b t', b=16)

        def mtile(tag, dt=F32):
            return sbm.tile([128, 512], dt, tag=tag, name=tag)
        P = mtile('P')
        T32 = mtile('T32', I32)
        Tf = mtile('Tf')
        Gg = mtile('Gg')

        with tc.tile_pool(name='convsb', bufs=1) as convsb, \
             tc.tile_pool(name='psA', bufs=1, space='PSUM') as psA, \
             tc.tile_pool(name='psI', bufs=2, space='PSUM') as psI, \
             tc.tile_pool(name='psT', bufs=2, space='PSUM') as psT, \
             tc.tile_pool(name='psW', bufs=2, space='PSUM') as psW:
            # ---- stage 1: upsample template + build combined [512ch, 34, 66]
            # (the fixed-scale mults run on GpSimd, idle at this point)
            comb = []
            with tc.tile_pool(name='upsb', bufs=1) as upsb:
                for cg in range(2):
                    tp = upsb.tile([128, 23 * 32], BF16, tag=f'tp{cg}', name=f'tp{cg}')
                    nc.sync.dma_start(tp[:], tplp[128 * cg:128 * (cg + 1), :])
                    tp3 = tp[:].rearrange('p (r w) -> p r w', r=23)
                    V = upsb.tile([128, 34 * 32], BF16, tag=f'vt{cg}', name=f'vt{cg}')
                    V3 = V[:].rearrange('p (r w) -> p r w', r=34)
                    tmp = upsb.tile([128, 34 * 32], BF16, tag=f'ut{cg}', name=f'ut{cg}')
                    tmp3 = tmp[:].rearrange('p (r w) -> p r w', r=34)
                    nc.vector.tensor_scalar_mul(tmp3[:, 0:16, :], tp3[:, 2:18, :], 0.25)
                    nc.scalar.activation(V3[:, 2:34:2, :], tp3[:, 1:17, :], ACTF.Identity, scale=0.75)
                    nc.vector.tensor_tensor(V3[:, 2:34:2, :], V3[:, 2:34:2, :], tmp3[:, 0:16, :], ALU.add)
                    nc.vector.tensor_scalar_mul(tmp3[:, 0:16, :], tp3[:, 1:17, :], 0.75)
                    nc.scalar.activation(V3[:, 1:33:2, :], tp3[:, 0:16, :], ACTF.Identity, scale=0.25)
                    nc.vector.tensor_tensor(V3[:, 1:33:2, :], V3[:, 1:33:2, :], tmp3[:, 0:16, :], ALU.add)
                    nc.vector.tensor_scalar_mul(tmp3[:, 0:1, :], tp3[:, 20:21, :], 0.25)
                    nc.scalar.activation(V3[:, 0:1, :], tp3[:, 19:20, :], ACTF.Identity, scale=0.75)
                    nc.vector.tensor_tensor(V3[:, 0:1, :], V3[:, 0:1, :], tmp3[:, 0:1, :], ALU.add)
                    nc.vector.tensor_scalar_mul(tmp3[:, 0:1, :], tp3[:, 22:23, :], 0.75)
                    nc.scalar.activation(V3[:, 33:34, :], tp3[:, 21:22, :], ACTF.Identity, scale=0.25)
                    nc.vector.tensor_tensor(V3[:, 33:34, :], V3[:, 33:34, :], tmp3[:, 0:1, :], ALU.add)
                    cb = convsb.tile([128, 34 * 66], BF16, tag=f'comb{cg}', name=f'comb{cg}')
                    cb3 = cb[:].rearrange('p (r w) -> p r w', r=34)
                    nc.vector.memset(cb[:], 0.0)
                    h3 = tmp3
                    nc.vector.tensor_scalar_mul(h3[:, :, 0:31], V3[:, :, 1:32], 0.75)
                    nc.scalar.activation(cb3[:, :, 3:65:2], V3[:, :, 0:31], ACTF.Identity, scale=0.25)
                    nc.vector.tensor_tensor(cb3[:, :, 3:65:2], cb3[:, :, 3:65:2], h3[:, :, 0:31], ALU.add)
                    nc.vector.tensor_scalar_mul(h3[:, :, 0:31], V3[:, :, 1:32], 0.25)
                    nc.scalar.activation(cb3[:, :, 2:64:2], V3[:, :, 0:31], ACTF.Identity, scale=0.75)
                    nc.vector.tensor_tensor(cb3[:, :, 2:64:2], cb3[:, :, 2:64:2], h3[:, :, 0:31], ALU.add)
                    nc.vector.tensor_copy(cb3[:, :, 1:2], V3[:, :, 0:1])
                    nc.vector.tensor_copy(cb3[:, :, 64:65], V3[:, :, 31:32])
                    comb.append(cb)
                for cg in range(2):
                    cb = convsb.tile([128, 34 * 66], BF16, tag=f'comb{cg+2}', name=f'comb{cg+2}')
                    nc.sync.dma_start(cb[:], srch66[128 * cg:128 * (cg + 1), :])
                    comb.append(cb)

            # ---- stage 2+3: per-quarter conv; per-half idx/wq chain ----
            wp4 = wp[:].rearrange('p (g t m) -> p g t m', g=4, t=9)
            for q in range(4):
                pomq = psA.tile([32, 512], F32, tag=f'pom{q % 2}', name=f'pom{q % 2}')
                first = True
                for gi, g in enumerate((2, 3, 0, 1)):
                    cb3 = comb[g][:].rearrange('p (r w) -> p r w', r=34)
                    for t, (ky, kx) in enumerate(TAPS):
                        rhs = cb3[:, 8 * q + 1 + ky: 8 * q + 9 + ky, 1 + kx: 65 + kx]
                        nc.tensor.matmul(
                            pomq[:], wp4[:, g, t, :], rhs,
                            start=first, stop=(gi == 3 and t == 8),
                            tile_position=(0, 0))
                        first = False
                ps = slice(32 * q, 32 * q + 32)
                nc.scalar.activation(om[ps, :], pomq[:], ACTF.Identity,
                                     bias=bom_sb[ps, :], scale=1.0)
                nc.scalar.activation(sg[ps, :], om[ps, :], ACTF.Sigmoid)

                if q % 2 == 0:
                    continue
                hq = q // 2            # half index: quarters 2hq, 2hq+1
                hs = slice(64 * hq, 64 * hq + 64)
                # index math for this half
                nc.vector.tensor_tensor(P[hs, :], om[hs, :], base_sb[hs, :], ALU.add)
                nc.vector.tensor_scalar(P[hs, :], P[hs, :], CLAMP_HI, CLAMP_LO,
                                        ALU.min, ALU.max)
                nc.vector.tensor_copy(T32[hs, :], P[hs, :])
                nc.vector.tensor_copy(Tf[hs, :], T32[hs, :])
                nc.vector.tensor_tensor(Gg[hs, :], Tf[hs, :], P[hs, :], ALU.is_gt)
                nc.vector.tensor_tensor(Tf[hs, :], Tf[hs, :], Gg[hs, :], ALU.subtract)
                nc.vector.tensor_tensor(P[hs, :], P[hs, :], Tf[hs, :], ALU.subtract)
                Wf = P
                for qq in (2 * hq, 2 * hq + 1):
                    qs = slice(32 * qq, 32 * qq + 32)
                    # idxf: 66*(y0+1) + (x0+1)
                    pidx = psI.tile([9, 512], F32, name='pidx')
                    nc.tensor.matmul(pidx[:], mc_sb[qs, 9 * qq:9 * (qq + 1)],
                                     Tf[qs, :], start=True, stop=True,
                                     tile_position=(32 * qq, 0))
                    nc.vector.tensor_scalar(idxf[:, 512 * qq:512 * (qq + 1)], pidx[:],
                                            67.0, None, ALU.add)
                    # wq transposes for this quarter's four 128-px blocks
                    for cc in range(4):
                        bb = 4 * qq + cc
                        pt = psW.tile([128, 45], F32, name='ptw')
                        idq18 = identf[32 * qq:32 * qq + 18, 32 * qq:32 * qq + 18]
                        idq27 = identf[32 * qq:32 * qq + 27, 32 * qq:32 * qq + 27]
                        nc.tensor.transpose(
                            pt[:, 0:18],
                            Wf[32 * qq:32 * qq + 18, 128 * cc:128 * (cc + 1)],
                            idq18, tile_position=(32 * qq, 0))
                        nc.tensor.transpose(
                            pt[:, 18:45],
                            sg[32 * qq:32 * qq + 27, 128 * cc:128 * (cc + 1)],
                            idq27, tile_position=(32 * qq, 0))
                        nc.vector.tensor_copy(ta4[:, bb, :], pt[:])
                # idx16 wrap for this half
                for bb in range(8 * hq, 8 * hq + 8):
                    pT = psT.tile([128, 9], F32, tag='pTm', name='pT')
                    nc.tensor.transpose(pT[:], idxf[:, 128 * bb:128 * (bb + 1)],
                                        identf[0:9, 0:9], tile_position=(0, 0))
                    nc.vector.tensor_copy(tsb3[:, bb, :], pT[:])
                for a in range(8):
                    pWm = psT.tile([16, 72], F32, tag='pTm', name='pWm')
                    nc.tensor.matmul(pWm[:], identf[:, 16 * a:16 * (a + 1)],
                                     tsb[:, 72 * hq:72 * (hq + 1)],
                                     start=True, stop=True)
                    dsta = AP(idx16[:].tensor, idx16[:].offset + 64 * hq + a,
                              [[9 * 128, 16], [8, 8], [128, 9]])
                    nc.vector.tensor_copy(
                        dsta, pWm[:].rearrange('p (b t) -> p b t', b=8))
                # replicate idx rows 0:16 across all 128 partitions
                for (dst, src, npart) in ((16, 0, 16), (32, 0, 32), (64, 0, 64)):
                    d = AP(idx16[:].tensor, idx16[:].offset + dst * 9 * 128 + 64 * hq,
                           [[9 * 128, npart], [128, 9], [1, 64]])
                    s = AP(idx16[:].tensor, idx16[:].offset + src * 9 * 128 + 64 * hq,
                           [[9 * 128, npart], [128, 9], [1, 64]])
                    nc.sync.dma_start(d, s)
                # corner-weight products for this half's 8 blocks
                bs = slice(8 * hq, 8 * hq + 8)
                wy_ap = ta4[:, bs, 0:9]
                wx_ap = ta4[:, bs, 9:18]
                mk_ap = ta4[:, bs, 36:45]
                nc.vector.tensor_scalar(iw4[:, bs, 0, :], wy_ap, -1.0, 1.0,
                                        ALU.mult, ALU.add)
                nc.vector.tensor_scalar(iw4[:, bs, 1, :], wx_ap, -1.0, 1.0,
                                        ALU.mult, ALU.add)
                nc.vector.tensor_tensor(q13[:, bs, :], iw4[:, bs, 0, :], mk_ap, ALU.mult)
                nc.vector.tensor_tensor(q23[:, bs, :], wy_ap, mk_ap, ALU.mult)
                nc.vector.tensor_tensor(wq4[:, bs, 0, :], q13[:, bs, :],
                                        iw4[:, bs, 1, :], ALU.mult)
                nc.vector.tensor_tensor(wq4[:, bs, 1, :], q13[:, bs, :], wx_ap, ALU.mult)
                nc.vector.tensor_tensor(wq4[:, bs, 2, :], q23[:, bs, :],
                                        iw4[:, bs, 1, :], ALU.mult)
                nc.vector.tensor_tensor(wq4[:, bs, 3, :], q23[:, bs, :], wx_ap, ALU.mult)

        # ---- stages 4-7 ----
        inap = AP(qtab[:].tensor, 0, [[1024, 4356], [1, 1024]])
        dw4 = dw[:].rearrange('p (k g o c) -> p k g o c', k=9, g=2, o=2)

        def chain_A(g3, sblk, S3, bb, t):
            # 1 ACT start + 3 DVE fused mult-adds
            m = sba.tile([128, 256], BF16, tag='m', name='m')
            nc.scalar.activation(m[:], g3[:, sblk, 0:256], ACTF.Identity,
                                 scale=wq4[:, bb, 0, t:t + 1])
            nc.vector.scalar_tensor_tensor(
                m[:], g3[:, sblk, 256:512], wq4[:, bb, 1, t:t + 1], m[:],
                ALU.mult, ALU.add)
            nc.vector.scalar_tensor_tensor(
                m[:], g3[:, sblk, 512:768], wq4[:, bb, 2, t:t + 1], m[:],
                ALU.mult, ALU.add)
            nc.vector.scalar_tensor_tensor(
                S3[:, sblk, :], g3[:, sblk, 768:1024], wq4[:, bb, 3, t:t + 1], m[:],
                ALU.mult, ALU.add)

        def chain_B(g3, sblk, S3, bb, t):
            # 2 ACT starts + 2 DVE fused mult-adds + 1 DVE pair-sum
            m1 = sbg.tile([128, 256], BF16, tag='mb1', name='mb1')
            m2 = sbg.tile([128, 256], BF16, tag='mb2', name='mb2')
            nc.scalar.activation(m1[:], g3[:, sblk, 0:256], ACTF.Identity,
                                 scale=wq4[:, bb, 0, t:t + 1])
            nc.vector.scalar_tensor_tensor(
                m1[:], g3[:, sblk, 256:512], wq4[:, bb, 1, t:t + 1], m1[:],
                ALU.mult, ALU.add)
            nc.scalar.activation(m2[:], g3[:, sblk, 512:768], ACTF.Identity,
                                 scale=wq4[:, bb, 2, t:t + 1])
            nc.vector.scalar_tensor_tensor(
                S3[:, sblk, :], g3[:, sblk, 768:1024], wq4[:, bb, 3, t:t + 1], m2[:],
                ALU.mult, ALU.add)
            nc.vector.tensor_tensor(S3[:, sblk, :], S3[:, sblk, :], m1[:], ALU.add)

        with tc.tile_pool(name='psQ', bufs=2, space='PSUM') as psQ, \
             tc.tile_pool(name='psO', bufs=2, space='PSUM') as psO, \
             tc.tile_pool(name='psO1', bufs=2, space='PSUM') as psO1:
            for qo in range(4):
                ST = stpool.tile([128, 18 * 512], BF16, tag='ST', name='ST')
                ST3 = ST[:].rearrange('p (k n) -> p k n', k=18)
                po0 = psO.tile([128, 512], F32, tag='po0', name='po0')
                Ss = {}

                def do_transposes(ptap):
                    pS3 = Ss[ptap]
                    pq = psQ.tile([128, 1024], BF16, name='pq')
                    for cg in range(2):
                        for sblk in range(4):
                            nc.tensor.transpose(
                                pq[:, 512 * cg + 128 * sblk:512 * cg + 128 * (sblk + 1)],
                                pS3[:, sblk, 128 * cg:128 * (cg + 1)], ident[:])
                    nc.scalar.activation(
                        ST3[:, 2 * ptap:2 * ptap + 2, :],
                        pq[:].rearrange('p (c n) -> p c n', c=2), ACTF.Identity)
                    for cg in range(2):
                        nc.tensor.matmul(
                            po0[:], dw4[:, ptap, cg, 0, :], ST3[:, 2 * ptap + cg, :],
                            start=(ptap == 0 and cg == 0), stop=(ptap == 8 and cg == 1))

                for t in range(9):
                    g = gpool.tile([128, 4, 1024], BF16, tag='gt', name='gt')
                    nc.gpsimd.dma_gather(
                        out_ap=g[:], in_ap=inap,
                        idxs_ap=idx16[:, 128 * t + 32 * qo:128 * t + 32 * (qo + 1)],
                        num_idxs=512, num_idxs_reg=512,
                        elem_size=1024, elem_step=1024,
                        single_packet=False,
                        queue_num=(9 * qo + t) % 4)
                    g3 = g[:]
                    S = spool.tile([128, 4, 256], BF16, tag='S', name='S')
                    S3 = S[:]
                    Ss[t] = S3
                    for sblk in range(4):
                        bb = 4 * qo + sblk
                        if sblk == 3:
                            chain_B(g3, sblk, S3, bb, t)
                        else:
                            chain_A(g3, sblk, S3, bb, t)
                    if t > 0:
                        do_transposes(t - 1)
                do_transposes(8)
                osb = sbt.tile([128, 512], F32, tag='osb', name='osb')
                nc.scalar.activation(osb[:], po0[:], ACTF.Identity,
                                     bias=bout_sb[:, 0:1], scale=1.0)
                nc.sync.dma_start(out[0:128, 512 * qo:512 * (qo + 1)], osb[:])
                po = psO1.tile([128, 512], F32, name='po')
                for t in range(9):
                    for cg in range(2):
                        nc.tensor.matmul(
                            po[:], dw4[:, t, cg, 1, :], ST3[:, 2 * t + cg, :],
                            start=(t == 0 and cg == 0), stop=(t == 8 and cg == 1))
                osb1 = sbt.tile([128, 512], F32, tag='osb1', name='osb1')
                nc.scalar.activation(osb1[:], po[:], ACTF.Identity,
                                     bias=bout_sb[:, 1:2], scale=1.0)
                nc.sync.dma_start(out[128:256, 512 * qo:512 * (qo + 1)], osb1[:])

    nc.compile()
    return nc


def _bf16(x):
    import ml_dtypes
    return np.asarray(x, dtype=np.float32).astype(ml_dtypes.bfloat16)


def prep_sample(inputs, b):
    """Per-sample (shared by both h-halves) heavy prep: the quad table."""
    sf = np.ascontiguousarray(np.asarray(inputs['search_feat'][b], dtype=np.float32))
    P = np.zeros((67, 67, 256), np.float32)
    P[1:65, 1:65] = sf.transpose(1, 2, 0)
    Q = np.concatenate([P[:66, :66], P[:66, 1:67], P[1:67, :66], P[1:67, 1:67]],
                       axis=-1)
    return _bf16(Q.reshape(4356, 1024))


def prep_core_inputs(inputs, b, h, qtab):
    tf = np.ascontiguousarray(np.asarray(inputs['template_feat'][b], dtype=np.float32))
    sf = np.ascontiguousarray(np.asarray(inputs['search_feat'][b], dtype=np.float32))
    offset_w = np.asarray(inputs['offset_w'], dtype=np.float32)
    offset_b = np.asarray(inputs['offset_b'], dtype=np.float32)
    mask_w = np.asarray(inputs['mask_w'], dtype=np.float32)
    mask_b = np.asarray(inputs['mask_b'], dtype=np.float32)
    deform_w = np.asarray(inputs['deform_w'], dtype=np.float32)
    deform_b = np.asarray(inputs['deform_b'], dtype=np.float32)

    tplp = np.zeros((256, 23, 32), np.float32)
    for j in range(19):
        tplp[:, j] = tf[:, min(max(16 * h - 1 + j, 0), 31)]
    if h == 0:
        tplp[:, 21] = tf[:, 15]
        tplp[:, 22] = tf[:, 16]
    else:
        tplp[:, 19] = tf[:, 15]
        tplp[:, 20] = tf[:, 16]

    srch66 = np.zeros((256, 34, 66), np.float32)
    for i in range(34):
        r = 32 * h - 1 + i
        if 0 <= r <= 63:
            srch66[:, i, 1:65] = sf[:, r]

    wpack = np.zeros((128, 4, 9, 32), np.float32)
    for g in range(4):
        for t, (ky, kx) in enumerate(TAPS):
            cs = slice(128 * g, 128 * (g + 1))
            wpack[:, g, t, 0:9] = offset_w[0::2, cs, ky + 1, kx + 1].T
            wpack[:, g, t, 9:18] = offset_w[1::2, cs, ky + 1, kx + 1].T
            if ky == 0 and kx == 0:
                wpack[:, g, t, 18:27] = mask_w[:, cs, 0, 0].T
    wk = deform_w.reshape(256, 256, 3, 3)
    dwp = np.zeros((128, 9, 2, 2, 128), np.float32)
    for t in range(9):
        ky, kx = TAPS[t]
        for cg in range(2):
            for og in range(2):
                dwp[:, t, cg, og, :] = wk[128 * og:128 * (og + 1),
                                          128 * cg:128 * (cg + 1), ky + 1, kx + 1].T

    basei = np.zeros((128, 512), np.float32)
    col = np.arange(512)
    for q in range(4):
        for m in range(9):
            basei[32 * q + m] = 32 * h + 8 * q + col // 64 + TAPS[m][0]
            basei[32 * q + 9 + m] = col % 64 + TAPS[m][1]

    mcomb = np.zeros((128, 4, 9), np.float32)
    for q in range(4):
        for t in range(9):
            mcomb[32 * q + t, q, t] = 66.0
            mcomb[32 * q + 9 + t, q, t] = 1.0

    bias_om = np.zeros((128, 1), np.float32)
    for q in range(4):
        bias_om[32 * q + 0:32 * q + 9, 0] = offset_b[0::2]
        bias_om[32 * q + 9:32 * q + 18, 0] = offset_b[1::2]
        bias_om[32 * q + 18:32 * q + 27, 0] = mask_b

    return {
        'tplp': _bf16(tplp.reshape(256, 23 * 32)),
        'srch66': _bf16(srch66.reshape(256, 34 * 66)),
        'qtab': qtab,
        'wpack': _bf16(wpack.reshape(128, 4 * 9 * 32)),
        'dwpack': _bf16(dwp.reshape(128, 9 * 2 * 2 * 128)),
        'basei': basei,
        'mcomb': mcomb.reshape(128, 4 * 9),
        'bias_om': bias_om,
        'bias_out': deform_b.reshape(256, 1).astype(np.float32),
    }


def kernel(**inputs):
    key = 'v41'
    if key not in _NC_CACHE:
        _NC_CACHE[key] = build_nc()
    nc = _NC_CACHE[key]
    qtabs = [prep_sample(inputs, b) for b in range(4)]
    in_maps = [prep_core_inputs(inputs, ci // 2, ci % 2, qtabs[ci // 2])
               for ci in range(8)]
    res = run_bass_kernel_spmd(nc, in_maps, core_ids=list(range(8)))
    global LAST_RESULT
    LAST_RESULT = res
    out = np.zeros((4, 256, 64, 64), np.float32)
    for ci in range(8):
        b, h = ci // 2, ci % 2
        out[b][:, 32 * h:32 * h + 32, :] = res.results[ci]['out'].reshape(256, 32, 64)
    return out


# revision 33
# speedup vs baseline: 3.1909x; 1.0102x over previous
"""Self-contained Trainium2 kernel for nn_DynamicCrossAttention_40286793236903.

kernel(**inputs) takes the FULL inputs (as produced by setup_inputs) and
returns the FULL [4, 256, 64, 64] float32 output.

Sharding: pure data parallel over (batch, image-half): core ci handles
sample b=ci//2, output rows 32*(ci%2)..32*(ci%2)+31. One SPMD Bass program
runs on all 8 cores; all per-core variation is carried in the input data.

Pipeline per core (2048 output pixels = 4 quarters of 512):
  1. bilinear-upsample template half (+halo) -> combined [512ch, 34, 66]
  2. offsets+mask 3x3 conv, per quarter q -> om[32q:32q+32, 512]
     (rows 32q+0..8 = dy taps, +9..17 = dx, +18..26 = mask)
  3. per half: index math (clamp to [-1, 64.996], floor/frac), quad-table
     idx = 66*(y0+1)+(x0+1), idx16 wrap for dma_gather, corner weights
     wq = bilinear products * sigmoid(mask)
  4. main loop per (quarter, tap): dma_gather 512 quad rows (2KB each: the
     4 bilinear corners, zero-padded at borders -> no validity masks);
     blend per 128-px block: chains of per-partition-scalar mult-adds
     spread over ACT (starts), DVE (fused mult-adds), GpSimd (pair sums,
     delayed one tap so they never stall the next gather's emission);
     PE-transpose to channel-major ST; einsum og=0 accumulated inline,
     og=1 as a per-quarter tail.
"""
import numpy as np
from contextlib import ExitStack

import concourse.bass as bass
import concourse.mybir as mybir
import concourse.tile as tile
from concourse import bacc
from concourse.bass import AP
from concourse.bass_utils import run_bass_kernel_spmd
from concourse.masks import make_identity

F32 = mybir.dt.float32
BF16 = mybir.dt.bfloat16
I32 = mybir.dt.int32
I16 = mybir.dt.int16
ALU = mybir.AluOpType
ACTF = mybir.ActivationFunctionType

TAPS = [(ky, kx) for ky in (-1, 0, 1) for kx in (-1, 0, 1)]
CLAMP_LO = -1.0
CLAMP_HI = 64.99609375

_NC_CACHE = {}
LAST_RESULT = None


def build_nc():
    nc = bacc.Bacc(None, target_bir_lowering=False, num_swdge_queues=4)

    tplp = nc.dram_tensor('tplp', [256, 23 * 32], BF16, kind='ExternalInput')
    srch66 = nc.dram_tensor('srch66', [256, 34 * 66], BF16, kind='ExternalInput')
    qtab = nc.dram_tensor('qtab', [4356, 1024], BF16, kind='ExternalInput')
    wpack = nc.dram_tensor('wpack', [128, 4 * 9 * 32], BF16, kind='ExternalInput')
    dwpack = nc.dram_tensor('dwpack', [128, 9 * 2 * 2 * 128], BF16, kind='ExternalInput')
    basei = nc.dram_tensor('basei', [128, 512], F32, kind='ExternalInput')
    mcomb = nc.dram_tensor('mcomb', [128, 4 * 9], F32, kind='ExternalInput')
    bias_om = nc.dram_tensor('bias_om', [128, 1], F32, kind='ExternalInput')
    bias_out = nc.dram_tensor('bias_out', [256, 1], F32, kind='ExternalInput')
    out = nc.dram_tensor('out', [256, 2048], F32, kind='ExternalOutput')

    with tile.TileContext(nc) as tc, ExitStack() as ctx:
        sb = ctx.enter_context(tc.tile_pool(name='sb', bufs=1))
        sbm = ctx.enter_context(tc.tile_pool(name='sbm', bufs=1))
        sbt = ctx.enter_context(tc.tile_pool(name='sbt', bufs=2))
        sba = ctx.enter_context(tc.tile_pool(name='sba', bufs=4))
        sbg = ctx.enter_context(tc.tile_pool(name='sbg', bufs=6))
        gpool = ctx.enter_context(tc.tile_pool(name='gpool', bufs=8))
        spool = ctx.enter_context(tc.tile_pool(name='spool', bufs=3))
        stpool = ctx.enter_context(tc.tile_pool(name='stpool', bufs=2))

        ident = sb.tile([128, 128], BF16, tag='ident', name='ident')
        make_identity(nc, ident[:])
        identf = sb.tile([128, 128], F32, tag='identf', name='identf')
        make_identity(nc, identf[:])

        wp = sb.tile([128, 4 * 9 * 32], BF16, tag='wp', name='wp')
        nc.sync.dma_start(wp[:], wpack[:])
        dw = sb.tile([128, 9 * 2 * 2 * 128], BF16, tag='dw', name='dw')
        nc.sync.dma_start(dw[:], dwpack[:])
        base_sb = sb.tile([128, 512], F32, tag='base', name='base')
        nc.sync.dma_start(base_sb[:], basei[:])
        mc_sb = sb.tile([128, 4 * 9], F32, tag='mcomb', name='mcomb')
        nc.sync.dma_start(mc_sb[:], mcomb[:])
        bom_sb = sb.tile([128, 1], F32, tag='bom', name='bom')
        nc.sync.dma_start(bom_sb[:], bias_om[:])
        bout_sb = sb.tile([128, 2], F32, tag='bout', name='bout')
        nc.sync.dma_start(bout_sb[:], bias_out[:].rearrange('(g p) o -> p (g o)', g=2))

        # persistent tiles for the idx/weight pipeline
        om = sb.tile([128, 512], F32, tag='om', name='om')
        sg = sb.tile([128, 512], F32, tag='sg', name='sg')
        idxf = sb.tile([9, 2048], F32, tag='idxf', name='idxf')
        idx16 = sb.tile([128, 9 * 128], I16, tag='idx16', name='idx16')
        tsb = sb.tile([128, 16 * 9], F32, tag='tsb', name='tsb')
        tsb3 = tsb[:].rearrange('p (b t) -> p b t', b=16)
        wq = sb.tile([128, 16 * 4 * 9], F32, tag='wq', name='wq')
        wq4 = wq[:].rearrange('p (b j t) -> p b j t', b=16, j=4)
        taW = sb.tile([128, 16 * 45], F32, tag='taW', name='taW')
        ta4 = taW[:].rearrange('p (b c) -> p b c', b=16)
        iw = sb.tile([128, 16 * 2 * 9], F32, tag='iw', name='iw')
        iw4 = iw[:].rearrange('p (b j t) -> p b j t', b=16, j=2)
        q1t = sb.tile([128, 16 * 9], F32, tag='q1t', name='q1t')
        q13 = q1t[:].rearrange('p (b t) -> p b t', b=16)
        q2t = sb.tile([128, 16 * 9], F32, tag='q2t', name='q2t')
        q23 = q2t[:].rearrange('p (b t) -> p b t', b=16)

        def mtile(tag, dt=F32):
            return sbm.tile([128, 512], dt, tag=tag, name=tag)
        P = mtile('P')
        T32 = mtile('T32', I32)
        Tf = mtile('Tf')
        Gg = mtile('Gg')

        convsb = ctx.enter_context(tc.tile_pool(name='convsb', bufs=1))
        psA = ctx.enter_context(tc.tile_pool(name='psA', bufs=1, space='PSUM'))
        psI = ctx.enter_context(tc.tile_pool(name='psI', bufs=1, space='PSUM'))
        psT = ctx.enter_context(tc.tile_pool(name='psT', bufs=1, space='PSUM'))
        psW = ctx.enter_context(tc.tile_pool(name='psW', bufs=1, space='PSUM'))
        psQ = ctx.enter_context(tc.tile_pool(name='psQ', bufs=2, space='PSUM'))
        psO = ctx.enter_context(tc.tile_pool(name='psO', bufs=1, space='PSUM'))
        psO1 = ctx.enter_context(tc.tile_pool(name='psO1', bufs=1, space='PSUM'))

        if True:
            # ---- stage 1: upsample template + build combined [512ch, 34, 66]
            comb = []
            with tc.tile_pool(name='upsb', bufs=1) as upsb:
                for cg in range(2):
                    tp = upsb.tile([128, 23 * 32], BF16, tag=f'tp{cg}', name=f'tp{cg}')
                    nc.sync.dma_start(tp[:], tplp[128 * cg:128 * (cg + 1), :])
                    tp3 = tp[:].rearrange('p (r w) -> p r w', r=23)
                    V = upsb.tile([128, 34 * 32], BF16, tag=f'vt{cg}', name=f'vt{cg}')
                    V3 = V[:].rearrange('p (r w) -> p r w', r=34)
                    tmp = upsb.tile([128, 34 * 32], BF16, tag=f'ut{cg}', name=f'ut{cg}')
                    tmp3 = tmp[:].rearrange('p (r w) -> p r w', r=34)
                    nc.vector.tensor_scalar_mul(tmp3[:, 0:16, :], tp3[:, 2:18, :], 0.25)
                    nc.scalar.activation(V3[:, 2:34:2, :], tp3[:, 1:17, :], ACTF.Identity, scale=0.75)
                    nc.vector.tensor_tensor(V3[:, 2:34:2, :], V3[:, 2:34:2, :], tmp3[:, 0:16, :], ALU.add)
                    nc.vector.tensor_scalar_mul(tmp3[:, 0:16, :], tp3[:, 1:17, :], 0.75)
                    nc.scalar.activation(V3[:, 1:33:2, :], tp3[:, 0:16, :], ACTF.Identity, scale=0.25)
                    nc.vector.tensor_tensor(V3[:, 1:33:2, :], V3[:, 1:33:2, :], tmp3[:, 0:16, :], ALU.add)
                    nc.vector.tensor_scalar_mul(tmp3[:, 0:1, :], tp3[:, 20:21, :], 0.25)
                    nc.scalar.activation(V3[:, 0:1, :], tp3[:, 19:20, :], ACTF.Identity, scale=0.75)
                    nc.vector.tensor_tensor(V3[:, 0:1, :], V3[:, 0:1, :], tmp3[:, 0:1, :], ALU.add)
                    nc.vector.tensor_scalar_mul(tmp3[:, 0:1, :], tp3[:, 22:23, :], 0.75)
                    nc.scalar.activation(V3[:, 33:34, :], tp3[:, 21:22, :], ACTF.Identity, scale=0.25)
                    nc.vector.tensor_tensor(V3[:, 33:34, :], V3[:, 33:34, :], tmp3[:, 0:1, :], ALU.add)
                    cb = convsb.tile([128, 34 * 66], BF16, tag=f'comb{cg}', name=f'comb{cg}')
                    cb3 = cb[:].rearrange('p (r w) -> p r w', r=34)
                    nc.vector.memset(cb[:], 0.0)
                    h3 = tmp3
                    nc.vector.tensor_scalar_mul(h3[:, :, 0:31], V3[:, :, 1:32], 0.75)
                    nc.scalar.activation(cb3[:, :, 3:65:2], V3[:, :, 0:31], ACTF.Identity, scale=0.25)
                    nc.vector.tensor_tensor(cb3[:, :, 3:65:2], cb3[:, :, 3:65:2], h3[:, :, 0:31], ALU.add)
                    nc.vector.tensor_scalar_mul(h3[:, :, 0:31], V3[:, :, 1:32], 0.25)
                    nc.scalar.activation(cb3[:, :, 2:64:2], V3[:, :, 0:31], ACTF.Identity, scale=0.75)
                    nc.vector.tensor_tensor(cb3[:, :, 2:64:2], cb3[:, :, 2:64:2], h3[:, :, 0:31], ALU.add)
                    nc.vector.tensor_copy(cb3[:, :, 1:2], V3[:, :, 0:1])
                    nc.vector.tensor_copy(cb3[:, :, 64:65], V3[:, :, 31:32])
                    comb.append(cb)
                for cg in range(2):
                    cb = convsb.tile([128, 34 * 66], BF16, tag=f'comb{cg+2}', name=f'comb{cg+2}')
                    nc.sync.dma_start(cb[:], srch66[128 * cg:128 * (cg + 1), :])
                    comb.append(cb)

        # ---- stage 2+3 as emission helpers ----
        wp4 = wp[:].rearrange('p (g t m) -> p g t m', g=4, t=9)

        def conv_quarter(q):
            pomq = psA.tile([32, 512], F32, tag='pom', name='pom')
            first = True
            for gi, g in enumerate((2, 3, 0, 1)):
                cb3 = comb[g][:].rearrange('p (r w) -> p r w', r=34)
                for t, (ky, kx) in enumerate(TAPS):
                    rhs = cb3[:, 8 * q + 1 + ky: 8 * q + 9 + ky, 1 + kx: 65 + kx]
                    nc.tensor.matmul(
                        pomq[:], wp4[:, g, t, :], rhs,
                        start=first, stop=(gi == 3 and t == 8),
                        tile_position=(0, 0))
                    first = False
            ps = slice(32 * q, 32 * q + 32)
            nc.scalar.activation(om[ps, :], pomq[:], ACTF.Identity,
                                 bias=bom_sb[ps, :], scale=1.0)
            nc.scalar.activation(sg[ps, :], om[ps, :], ACTF.Sigmoid)

        def half_chain(hq):
            hs = slice(64 * hq, 64 * hq + 64)
            # index math for this half
            nc.vector.tensor_tensor(P[hs, :], om[hs, :], base_sb[hs, :], ALU.add)
            nc.vector.tensor_scalar(P[hs, :], P[hs, :], CLAMP_HI, CLAMP_LO,
                                    ALU.min, ALU.max)
            nc.vector.tensor_copy(T32[hs, :], P[hs, :])
            nc.vector.tensor_copy(Tf[hs, :], T32[hs, :])
            nc.vector.tensor_tensor(Gg[hs, :], Tf[hs, :], P[hs, :], ALU.is_gt)
            nc.vector.tensor_tensor(Tf[hs, :], Tf[hs, :], Gg[hs, :], ALU.subtract)
            nc.vector.tensor_tensor(P[hs, :], P[hs, :], Tf[hs, :], ALU.subtract)
            Wf = P
            for qq in (2 * hq, 2 * hq + 1):
                qs = slice(32 * qq, 32 * qq + 32)
                # idxf: 66*(y0+1) + (x0+1)
                pidx = psI.tile([9, 512], F32, name='pidx')
                nc.tensor.matmul(pidx[:], mc_sb[qs, 9 * qq:9 * (qq + 1)],
                                 Tf[qs, :], start=True, stop=True,
                                 tile_position=(32 * qq, 0))
                nc.vector.tensor_scalar(idxf[:, 512 * qq:512 * (qq + 1)], pidx[:],
                                        67.0, None, ALU.add)
                # wq transposes for this quarter's four 128-px blocks
                for cc in range(4):
                    bb = 4 * qq + cc
                    pt = psW.tile([128, 45], F32, name='ptw')
                    idq18 = identf[32 * qq:32 * qq + 18, 32 * qq:32 * qq + 18]
                    idq27 = identf[32 * qq:32 * qq + 27, 32 * qq:32 * qq + 27]
                    nc.tensor.transpose(
                        pt[:, 0:18],
                        Wf[32 * qq:32 * qq + 18, 128 * cc:128 * (cc + 1)],
                        idq18, tile_position=(32 * qq, 0))
                    nc.tensor.transpose(
                        pt[:, 18:45],
                        sg[32 * qq:32 * qq + 27, 128 * cc:128 * (cc + 1)],
                        idq27, tile_position=(32 * qq, 0))
                    nc.scalar.activation(ta4[:, bb, :], pt[:], ACTF.Identity)
            # idx16 wrap for this half
            for bb in range(8 * hq, 8 * hq + 8):
                pT = psT.tile([128, 9], F32, tag='pTm', name='pT')
                nc.tensor.transpose(pT[:], idxf[:, 128 * bb:128 * (bb + 1)],
                                    identf[0:9, 0:9], tile_position=(0, 0))
                nc.scalar.activation(tsb3[:, bb, :], pT[:], ACTF.Identity)
            for a in range(8):
                pWm = psT.tile([16, 72], F32, tag='pTm', name='pWm')
                nc.tensor.matmul(pWm[:], identf[:, 16 * a:16 * (a + 1)],
                                 tsb[:, 72 * hq:72 * (hq + 1)],
                                 start=True, stop=True)
                dsta = AP(idx16[:].tensor, idx16[:].offset + 64 * hq + a,
                          [[9 * 128, 16], [8, 8], [128, 9]])
                nc.vector.tensor_copy(
                    dsta, pWm[:].rearrange('p (b t) -> p b t', b=8))
            # replicate idx rows 0:16 across all 128 partitions
            for (dst, src, npart) in ((16, 0, 16), (32, 0, 32), (64, 0, 64)):
                d = AP(idx16[:].tensor, idx16[:].offset + dst * 9 * 128 + 64 * hq,
                       [[9 * 128, npart], [128, 9], [1, 64]])
                s = AP(idx16[:].tensor, idx16[:].offset + src * 9 * 128 + 64 * hq,
                       [[9 * 128, npart], [128, 9], [1, 64]])
                nc.sync.dma_start(d, s)
            # corner-weight products for this half's 8 blocks
            bs = slice(8 * hq, 8 * hq + 8)
            wy_ap = ta4[:, bs, 0:9]
            wx_ap = ta4[:, bs, 9:18]
            mk_ap = ta4[:, bs, 36:45]
            nc.vector.tensor_scalar(iw4[:, bs, 0, :], wy_ap, -1.0, 1.0,
                                    ALU.mult, ALU.add)
            nc.vector.tensor_scalar(iw4[:, bs, 1, :], wx_ap, -1.0, 1.0,
                                    ALU.mult, ALU.add)
            nc.vector.tensor_tensor(q13[:, bs, :], iw4[:, bs, 0, :], mk_ap, ALU.mult)
            nc.vector.tensor_tensor(q23[:, bs, :], wy_ap, mk_ap, ALU.mult)
            nc.vector.tensor_tensor(wq4[:, bs, 0, :], q13[:, bs, :],
                                    iw4[:, bs, 1, :], ALU.mult)
            nc.vector.tensor_tensor(wq4[:, bs, 1, :], q13[:, bs, :], wx_ap, ALU.mult)
            nc.vector.tensor_tensor(wq4[:, bs, 2, :], q23[:, bs, :],
                                    iw4[:, bs, 1, :], ALU.mult)
            nc.vector.tensor_tensor(wq4[:, bs, 3, :], q23[:, bs, :], wx_ap, ALU.mult)

        # ---- stages 4-7 ----
        inap = AP(qtab[:].tensor, 0, [[1024, 4356], [1, 1024]])
        dw4 = dw[:].rearrange('p (k g o c) -> p k g o c', k=9, g=2, o=2)

        def chain_A(g3, sblk, S3, bb, t):
            # 1 ACT start + 3 DVE fused mult-adds
            m = sba.tile([128, 256], BF16, tag='m', name='m')
            nc.scalar.activation(m[:], g3[:, sblk, 0:256], ACTF.Identity,
                                 scale=wq4[:, bb, 0, t:t + 1])
            nc.vector.scalar_tensor_tensor(
                m[:], g3[:, sblk, 256:512], wq4[:, bb, 1, t:t + 1], m[:],
                ALU.mult, ALU.add)
            nc.vector.scalar_tensor_tensor(
                m[:], g3[:, sblk, 512:768], wq4[:, bb, 2, t:t + 1], m[:],
                ALU.mult, ALU.add)
            nc.vector.scalar_tensor_tensor(
                S3[:, sblk, :], g3[:, sblk, 768:1024], wq4[:, bb, 3, t:t + 1], m[:],
                ALU.mult, ALU.add)

        def chain_B(g3, sblk, S3, bb, t):
            # 2 ACT starts + 2 DVE fused mult-adds + 1 DVE pair-sum
            m1 = sbg.tile([128, 256], BF16, tag='mb1', name='mb1')
            m2 = sbg.tile([128, 256], BF16, tag='mb2', name='mb2')
            nc.scalar.activation(m1[:], g3[:, sblk, 0:256], ACTF.Identity,
                                 scale=wq4[:, bb, 0, t:t + 1])
            nc.vector.scalar_tensor_tensor(
                m1[:], g3[:, sblk, 256:512], wq4[:, bb, 1, t:t + 1], m1[:],
                ALU.mult, ALU.add)
            nc.scalar.activation(m2[:], g3[:, sblk, 512:768], ACTF.Identity,
                                 scale=wq4[:, bb, 2, t:t + 1])
            nc.vector.scalar_tensor_tensor(
                S3[:, sblk, :], g3[:, sblk, 768:1024], wq4[:, bb, 3, t:t + 1], m2[:],
                ALU.mult, ALU.add)
            nc.vector.tensor_tensor(S3[:, sblk, :], S3[:, sblk, :], m1[:], ALU.add)

        def main_quarter(qo):
            ST = stpool.tile([128, 18 * 512], BF16, tag='ST', name='ST')
            ST3 = ST[:].rearrange('p (k n) -> p k n', k=18)
            po0 = psO.tile([128, 512], F32, tag='po0', name='po0')
            po1 = psO1.tile([128, 512], F32, tag='po1', name='po1')
            Ss = {}

            def do_transposes(ptap):
                pS3 = Ss[ptap]
                pq = psQ.tile([128, 1024], BF16, name='pq')
                for cg in range(2):
                    for sblk in range(4):
                        nc.tensor.transpose(
                            pq[:, 512 * cg + 128 * sblk:512 * cg + 128 * (sblk + 1)],
                            pS3[:, sblk, 128 * cg:128 * (cg + 1)], ident[:])
                nc.scalar.activation(
                    ST3[:, 2 * ptap:2 * ptap + 2, :],
                    pq[:].rearrange('p (c n) -> p c n', c=2), ACTF.Identity)
                for cg in range(2):
                    nc.tensor.matmul(
                        po0[:], dw4[:, ptap, cg, 0, :], ST3[:, 2 * ptap + cg, :],
                        start=(ptap == 0 and cg == 0), stop=(ptap == 8 and cg == 1))
                    nc.tensor.matmul(
                        po1[:], dw4[:, ptap, cg, 1, :], ST3[:, 2 * ptap + cg, :],
                        start=(ptap == 0 and cg == 0), stop=(ptap == 8 and cg == 1))

            for t in range(9):
                g = gpool.tile([128, 4, 1024], BF16, tag='gt', name='gt')
                nc.gpsimd.dma_gather(
                    out_ap=g[:], in_ap=inap,
                    idxs_ap=idx16[:, 128 * t + 32 * qo:128 * t + 32 * (qo + 1)],
                    num_idxs=512, num_idxs_reg=512,
                    elem_size=1024, elem_step=1024,
                    single_packet=False,
                    queue_num=(9 * qo + t) % 4)
                g3 = g[:]
                S = spool.tile([128, 4, 256], BF16, tag='S', name='S')
                S3 = S[:]
                Ss[t] = S3
                for sblk in range(4):
                    bb = 4 * qo + sblk
                    if sblk == 3:
                        chain_B(g3, sblk, S3, bb, t)
                    else:
                        chain_A(g3, sblk, S3, bb, t)
                if t > 0:
                    do_transposes(t - 1)
            do_transposes(8)
            osb = sbt.tile([128, 512], F32, tag='osb', name='osb')
            nc.scalar.activation(osb[:], po0[:], ACTF.Identity,
                                 bias=bout_sb[:, 0:1], scale=1.0)
            nc.sync.dma_start(out[0:128, 512 * qo:512 * (qo + 1)], osb[:])
            osb1 = sbt.tile([128, 512], F32, tag='osb1', name='osb1')
            nc.scalar.activation(osb1[:], po1[:], ACTF.Identity,
                                 bias=bout_sb[:, 1:2], scale=1.0)
            nc.sync.dma_start(out[128:256, 512 * qo:512 * (qo + 1)], osb1[:])

        # ---- emission order: keep each engine's FIFO free of cross-phase
        # stalls: half-1 prep work is emitted after quarter-0's main loop ----
        conv_quarter(0)
        conv_quarter(1)
        half_chain(0)
        conv_quarter(2)
        conv_quarter(3)
        main_quarter(0)
        half_chain(1)
        main_quarter(1)
        main_quarter(2)
        main_quarter(3)

    nc.compile()
    return nc


def _bf16(x):
    import ml_dtypes
    return np.asarray(x, dtype=np.float32).astype(ml_dtypes.bfloat16)


def prep_sample(inputs, b):
    """Per-sample (shared by both h-halves) heavy prep: the quad table."""
    sf = np.ascontiguousarray(np.asarray(inputs['search_feat'][b], dtype=np.float32))
    P = np.zeros((67, 67, 256), np.float32)
    P[1:65, 1:65] = sf.transpose(1, 2, 0)
    Q = np.concatenate([P[:66, :66], P[:66, 1:67], P[1:67, :66], P[1:67, 1:67]],
                       axis=-1)
    return _bf16(Q.reshape(4356, 1024))


def prep_core_inputs(inputs, b, h, qtab):
    tf = np.ascontiguousarray(np.asarray(inputs['template_feat'][b], dtype=np.float32))
    sf = np.ascontiguousarray(np.asarray(inputs['search_feat'][b], dtype=np.float32))
    offset_w = np.asarray(inputs['offset_w'], dtype=np.float32)
    offset_b = np.asarray(inputs['offset_b'], dtype=np.float32)
    mask_w = np.asarray(inputs['mask_w'], dtype=np.float32)
    mask_b = np.asarray(inputs['mask_b'], dtype=np.float32)
    deform_w = np.asarray(inputs['deform_w'], dtype=np.float32)
    deform_b = np.asarray(inputs['deform_b'], dtype=np.float32)

    tplp = np.zeros((256, 23, 32), np.float32)
    for j in range(19):
        tplp[:, j] = tf[:, min(max(16 * h - 1 + j, 0), 31)]
    if h == 0:
        tplp[:, 21] = tf[:, 15]
        tplp[:, 22] = tf[:, 16]
    else:
        tplp[:, 19] = tf[:, 15]
        tplp[:, 20] = tf[:, 16]

    srch66 = np.zeros((256, 34, 66), np.float32)
    for i in range(34):
        r = 32 * h - 1 + i
        if 0 <= r <= 63:
            srch66[:, i, 1:65] = sf[:, r]

    wpack = np.zeros((128, 4, 9, 32), np.float32)
    for g in range(4):
        for t, (ky, kx) in enumerate(TAPS):
            cs = slice(128 * g, 128 * (g + 1))
            wpack[:, g, t, 0:9] = offset_w[0::2, cs, ky + 1, kx + 1].T
            wpack[:, g, t, 9:18] = offset_w[1::2, cs, ky + 1, kx + 1].T
            if ky == 0 and kx == 0:
                wpack[:, g, t, 18:27] = mask_w[:, cs, 0, 0].T
    wk = deform_w.reshape(256, 256, 3, 3)
    dwp = np.zeros((128, 9, 2, 2, 128), np.float32)
    for t in range(9):
        ky, kx = TAPS[t]
        for cg in range(2):
            for og in range(2):
                dwp[:, t, cg, og, :] = wk[128 * og:128 * (og + 1),
                                          128 * cg:128 * (cg + 1), ky + 1, kx + 1].T

    basei = np.zeros((128, 512), np.float32)
    col = np.arange(512)
    for q in range(4):
        for m in range(9):
            basei[32 * q + m] = 32 * h + 8 * q + col // 64 + TAPS[m][0]
            basei[32 * q + 9 + m] = col % 64 + TAPS[m][1]

    mcomb = np.zeros((128, 4, 9), np.float32)
    for q in range(4):
        for t in range(9):
            mcomb[32 * q + t, q, t] = 66.0
            mcomb[32 * q + 9 + t, q, t] = 1.0

    bias_om = np.zeros((128, 1), np.float32)
    for q in range(4):
        bias_om[32 * q + 0:32 * q + 9, 0] = offset_b[0::2]
        bias_om[32 * q + 9:32 * q + 18, 0] = offset_b[1::2]
        bias_om[32 * q + 18:32 * q + 27, 0] = mask_b

    return {
        'tplp': _bf16(tplp.reshape(256, 23 * 32)),
        'srch66': _bf16(srch66.reshape(256, 34 * 66)),
        'qtab': qtab,
        'wpack': _bf16(wpack.reshape(128, 4 * 9 * 32)),
        'dwpack': _bf16(dwp.reshape(128, 9 * 2 * 2 * 128)),
        'basei': basei,
        'mcomb': mcomb.reshape(128, 4 * 9),
        'bias_om': bias_om,
        'bias_out': deform_b.reshape(256, 1).astype(np.float32),
    }


def kernel(**inputs):
    key = 'v41'
    if key not in _NC_CACHE:
        _NC_CACHE[key] = build_nc()
    nc = _NC_CACHE[key]
    qtabs = [prep_sample(inputs, b) for b in range(4)]
    in_maps = [prep_core_inputs(inputs, ci // 2, ci % 2, qtabs[ci // 2])
               for ci in range(8)]
    res = run_bass_kernel_spmd(nc, in_maps, core_ids=list(range(8)))
    global LAST_RESULT
    LAST_RESULT = res
    out = np.zeros((4, 256, 64, 64), np.float32)
    for ci in range(8):
        b, h = ci // 2, ci % 2
        out[b][:, 32 * h:32 * h + 32, :] = res.results[ci]['out'].reshape(256, 32, 64)
    return out
